# revision 56
# baseline (speedup 1.0000x reference)
"""Trainium2 Bass kernel for nn_MemoryModule (sparse_attention).

Reference computation (per batch b):
  Low branch:
    mkl (9216, 64) = memory_keys_low[b] as (T*Hl*Wl, Ck)
    qkl (64, 2304) = query_key_low[b]
    A = softmax_over_n(mkl @ qkl * Ck^-0.5)          # (9216, 2304)
    memory = mvl @ A                                  # (128, 2304)
  High branch:
    g_attn[t] = softmax_over_t(gk[t] @ gv[t].T * Cv^-0.5)   # (Ck, Cv) per t
    qout[t] = g_attn[t] @ qv                          # (64, 576) -> (256, 24, 24)
    qout = bilinear_upsample_2x(qout)                 # (256, 48, 48)
  out = concat([qout, memory.reshape(128, 48, 48)])   # (384, 48, 48)

Sharding: 8 cores = (b in 0..1) x (j in 0..3), j picks 576 of the 2304
low-branch query columns (= 12 of the 48 output rows). Softmax is over the
key axis, so column blocks are independent -> no collectives.

Implementation notes (fp8 + engine-split exp + two-bank column layout):
 - The 576 m-columns are laid out as (2, 288): half h lives in PSUM bank h
   of each 2-bank tile; every matmul target is a 288-col within-bank write.
 - Low branch entirely in fp8e4 (IEEE e4m3). Softmax is shift-invariant, so
   logits are shifted by -SIGMA before exp to stay in fp8 range.
 - QK: plain fp8 (contraction 64). DoubleRow QK saves no columns and its
   doubled MAC rate trips the chip's power throttle (50% PE clock cap).
   AV/denominator: DoubleRow over n-tile PAIRS (256-deep contraction) --
   halves their PE column count; throttle tolerates this mix (~13%).
 - exp split across ACT (exact exp -> fp8, 6/8 tiles) and DVE (uint8 bit
   trick: u8 = x*log2e + BIT_B IS the fp8 pattern of exp octave-linearized,
   2/8 tiles). Keeping DVE share low keeps chip power under the clamp
   threshold that otherwise halves the PE clock on 2-3 of the 8 cores.
 - High branch (bf16, precision-critical) interleaves through the same
   2-buffer qk PSUM pool; its softmax chain runs on the Pool engine.
 - PE clock (HAM gate): 1.2 GHz until one fully-busy 3.4us window of
   back-to-back matmuls, then 2.4 GHz; drops back if utilization sags.
   The normal loop's per-pair semaphore waits NEVER flip it. So: a 9-MM
   dummy warmup at pair WARM_AT flips it at ~20us, an av/dn backlog held
   back during pairs 0..WARM_AT-1 is flushed right after (keeps the PE
   gapless through the pipeline respool so the flip sticks), and qvup's
   dense matmul block at pair 6 re-flips any core that dropped on an
   unlucky window phase.
"""

import os
import sys

for _p in ("/opt/trn_rl_repo",):
    if _p not in sys.path and os.path.isdir(_p):
        sys.path.insert(0, _p)

import numpy as np
import ml_dtypes

import concourse.bass as bass
import concourse.tile as tile
from concourse import bacc, mybir
from concourse import bass_utils

BF16 = mybir.dt.bfloat16
F32 = mybir.dt.float32
F8 = mybir.dt.float8e4
U8 = mybir.dt.uint8

B, T, Ck, Cv = 2, 4, 64, 128
H, W, Hl, Wl = 24, 24, 48, 48
HW = H * W            # 576
NLOW = T * Hl * Wl    # 9216
MTOT = Hl * Wl        # 2304
MBLK = MTOT // 4      # 576 query columns per core
MH = MBLK // 2        # 288 columns per PSUM bank
NT = NLOW // 128      # 72 n-tiles
NPAIR = NT // 2       # 36 DoubleRow pairs
HWP = 640             # 576 padded to 5*128
NC_CHUNKS = HWP // 128  # 5

SCALE_LOW = float(Ck) ** -0.5   # 0.125
SCALE_HIGH = float(Cv) ** -0.5  # 0.0883883...

# fp8 exp range management: compute exp(s - SIGMA); shift cancels in softmax.
SIGMA = 1.25
LOG2E = 1.4426950408889634
# uint8 bit trick: u8 = round(x * BIT_C + BIT_B) has the fp8e4 bit pattern of
# approx exp(0.125*x - SIGMA).  (0.125*8*log2e = log2e; bias 56 = bits of 1.0;
# -0.344 centers the octave-linear interpolation error.)
BIT_C = LOG2E
BIT_B = 56.0 - 8.0 * SIGMA * LOG2E - 0.344

# exp engine assignment pattern, per n-tile index (cycled):
#   A=ACT exact, D=DVE bit trick  (Pool cannot read PSUM)
EXP_PATTERN = os.environ.get("K_EXP_PATTERN", "DA")
# single-matmul (2, 288) two-bank outputs: rejected by ISA (s3d3_mm_num_elements)
MM2B = os.environ.get("K_MM2B", "0") == "1"
# DMA qout rows straight from PSUM (rejected by bass: DMA src must be SBUF)
QO_DMA_PSUM = os.environ.get("K_QO_DMA_PSUM", "0") == "1"
# number of PE warm-up matmuls before the loop (ramps DVFS during DMA wait)
WARMUP_MM = int(os.environ.get("K_WARMUP_MM", "0"))
# mid-loop warmup: N dummy MMs emitted before pair K to flip the HAM clock
# gate at ~t(K) + 3.4us (late enough to not anger the power arbiter)
WARM_MID = int(os.environ.get("K_WARM_MID", "9"))
WARM_AT = int(os.environ.get("K_WARM_AT", "4"))
# denominator half-1 accumulated on the Pool engine: Pool decodes fp8 WRONG
DN_POOL = os.environ.get("K_DN_POOL", "0") == "1"
# first K pairs: dn half-1 accumulated on DVE instead of a PE matmul (PE
# relief during the slow-clock pre-flip phase, where DVE has slack)
DN_DVE_PAIRS = int(os.environ.get("K_DN_DVE", "0"))
# tail: tensor_tensor divide is not a valid DVE ISA op; keep reciprocal path
DIV_TAIL = os.environ.get("K_DIV_TAIL", "0") == "1"

_PROGRAM = None
LAST_PERF = {}


def _u1d(n_in, n_out):
    """Half-pixel bilinear interpolation matrix (n_out, n_in), matches
    jax.image.resize(method='bilinear') for upsampling."""
    U = np.zeros((n_out, n_in), dtype=np.float64)
    scale = n_in / n_out
    for i in range(n_out):
        c = (i + 0.5) * scale - 0.5
        f = int(np.floor(c))
        frac = c - f
        lo = min(max(f, 0), n_in - 1)
        hi = min(max(f + 1, 0), n_in - 1)
        U[i, lo] += 1.0 - frac
        U[i, hi] += frac
    return U


def _build_upsample_full():
    """(H*W, Hl*Wl): column (ho*Wl+wo), row (h*W+w)."""
    Uh = _u1d(H, Hl)  # (48, 24)
    Uw = _u1d(W, Wl)  # (48, 24)
    Ufull = np.einsum("oh,pw->hwop", Uh, Uw).reshape(H * W, Hl * Wl)
    return Ufull.astype(np.float32)


def _build_program():
    nc = bacc.Bacc("TRN2", target_bir_lowering=False, debug=False)

    d_qkl2 = nc.dram_tensor("qkl2", (64, 2, MH), F8, kind="ExternalInput")
    d_mk = nc.dram_tensor("mk", (64, NT, 128), F8, kind="ExternalInput")
    d_mvT = nc.dram_tensor("mvT", (128, NT, 128), F8, kind="ExternalInput")
    d_gkT = nc.dram_tensor("gkT", (128, T, NC_CHUNKS, Ck), BF16, kind="ExternalInput")
    d_gvT = nc.dram_tensor("gvT", (128, T, NC_CHUNKS, Cv), BF16, kind="ExternalInput")
    d_qvT = nc.dram_tensor("qvT", (128, NC_CHUNKS, Cv), BF16, kind="ExternalInput")
    d_uj = nc.dram_tensor("uj", (128, NC_CHUNKS, 2, MH), BF16, kind="ExternalInput")
    d_out = nc.dram_tensor("out", (T * Ck + Cv, 2, MH), BF16, kind="ExternalOutput")

    EXP = mybir.ActivationFunctionType.Exp
    DR = mybir.MatmulPerfMode.DoubleRow
    MUL = mybir.AluOpType.mult
    ADD = mybir.AluOpType.add

    with tile.TileContext(nc) as tc:
        from contextlib import ExitStack

        with ExitStack() as ctx:
            cp = ctx.enter_context(tc.tile_pool(name="const", bufs=1))
            wp = ctx.enter_context(tc.tile_pool(name="work", bufs=1))

            # qkl2 columns viewed as (k64, 2 m-halves, 288)
            qkl2_t = cp.tile([64, 2, MH], F8)
            mk_t = cp.tile([64, NT, 128], F8)
            mvT_t = cp.tile([128, NT, 128], F8)
            gkT_t = cp.tile([128, T, NC_CHUNKS, Ck], BF16)
            gvT_t = cp.tile([128, T, NC_CHUNKS, Cv], BF16)
            qvT_t = cp.tile([128, NC_CHUNKS, Cv], BF16)
            uj_t = cp.tile([128, NC_CHUNKS, 2, MH], BF16)

            # ---- DMA issue: scalar queue handles the first low-branch inputs
            # (it is idle until the first exp), sync queue streams the rest.
            # Small first chunks so qk0's inputs land ASAP; big high-branch
            # tensors go AFTER the first mk/mvT chunks to not hog bandwidth.
            nc.sync.dma_start(mk_t[:, 0:4, :], d_mk.ap()[:, 0:4, :])
            nc.scalar.dma_start(qkl2_t[:], d_qkl2.ap()[:, :, :])
            # mvT[0:4] before mk[4:16]: the warmup-seam av/dn flush needs it;
            # a late arrival gaps the PE right after the flip and drops the
            # clock gate on every core (+15us).
            nc.scalar.dma_start(mvT_t[:, 0:4, :], d_mvT.ap()[:, 0:4, :])
            nc.scalar.dma_start(mk_t[:, 4:16, :], d_mk.ap()[:, 4:16, :])
            nc.scalar.dma_start(mvT_t[:, 4:16, :], d_mvT.ap()[:, 4:16, :])
            nc.sync.dma_start(mk_t[:, 16:32, :], d_mk.ap()[:, 16:32, :])
            nc.sync.dma_start(mvT_t[:, 16:32, :], d_mvT.ap()[:, 16:32, :])
            nc.sync.dma_start(gvT_t[:], d_gvT.ap()[:, :, :, :])
            nc.sync.dma_start(gkT_t[:], d_gkT.ap()[:, :, :, :])
            nc.sync.dma_start(qvT_t[:], d_qvT.ap()[:, :, :])
            nc.sync.dma_start(uj_t[:], d_uj.ap()[:, :, :, :])
            nc.sync.dma_start(mk_t[:, 32:72, :], d_mk.ap()[:, 32:72, :])
            nc.sync.dma_start(mvT_t[:, 32:72, :], d_mvT.ap()[:, 32:72, :])

            ones8 = cp.tile([128, 2, 128], F8)
            nc.gpsimd.memset(ones8[:], 1.0)
            # per-partition scalar bias for the ACT exp path
            sig_t = cp.tile([128, 1], F32)
            nc.gpsimd.memset(sig_t[:], -SIGMA)

            with tc.tile_pool(name="qkps", bufs=2, space="PSUM") as qkps, \
                 tc.tile_pool(name="avps", bufs=1, space="PSUM") as avps, \
                 tc.tile_pool(name="dnps", bufs=1, space="PSUM") as dnps, \
                 tc.tile_pool(name="epool", bufs=8) as epool:

                # 2-bank accumulators; only the first MH columns of each bank
                # (plane) are used: column m = h*MH + c lives at [h, c].
                av = avps.tile([128, 2, 512], F32)
                dn = dnps.tile([128, 2, 512], F32)
                dn_h0 = dn[:, 0, 0:MH]

                def mm2b(out3, lhsT, rhs4, **kw):
                    """matmul into a (2, MH) two-bank output view."""
                    if MM2B:
                        nc.tensor.matmul(out3[:, :, 0:MH], lhsT, rhs4, **kw)
                    else:
                        for h in range(2):
                            nc.tensor.matmul(
                                out3[:, h, 0:MH], lhsT, rhs4[:, h, :], **kw)

                def mm2b_dr(out3, lhsT, rhs4, **kw):
                    """DoubleRow matmul into a (2, MH) two-bank output view.
                    rhs4 free dims: (2 k-planes, 2 m-halves, MH)."""
                    if MM2B:
                        nc.tensor.matmul(out3[:, :, 0:MH], lhsT, rhs4,
                                         perf_mode=DR, **kw)
                    else:
                        for h in range(2):
                            nc.tensor.matmul(
                                out3[:, h, 0:MH], lhsT, rhs4[:, :, h, :],
                                perf_mode=DR, **kw)

                def emit_qk(q):
                    """Plain fp8 QK for n-tile q (contraction 64, full MAC
                    rate, no DoubleRow -> no power-throttle trigger)."""
                    qk = qkps.tile([128, 2, 512], F32, name=f"qk{q}", tag="qk")
                    for h in range(2):
                        nc.tensor.matmul(
                            qk[:, h, 0:MH], mk_t[:, q, :], qkl2_t[:, h, :],
                            start=True, stop=True)
                    return qk

                def exp_write(e8, plane, qk, ti):
                    """e8[:, plane] = fp8(exp(0.125*qk - SIGMA)), (2, MH)."""
                    eng = EXP_PATTERN[ti % len(EXP_PATTERN)]
                    dst = e8[:, plane, :, :]
                    src = qk[:, :, 0:MH]
                    if eng == "A":
                        nc.scalar.activation(dst, src, EXP,
                                             bias=sig_t[:], scale=SCALE_LOW)
                    else:
                        nc.vector.tensor_scalar(
                            dst.bitcast(U8), src, BIT_C, BIT_B, MUL, ADD)

                # ---------- high-branch stages (bf16), emitted on demand ----
                hstate = {}

                def high_ga(trange):
                    for t in trange:
                        ga = qkps.tile([128, 2, 512], F32, name=f"ga{t}", tag="qk")
                        for c in range(NC_CHUNKS):
                            nc.tensor.matmul(
                                ga[:, 0, 0:Ck],
                                gvT_t[:, t, c, :],
                                gkT_t[:, t, c, :],
                                start=(c == 0),
                                stop=(c == NC_CHUNKS - 1),
                            )
                        e = wp.tile([128, Ck], F32, name=f"ea{t}", tag=f"ea{t}")
                        nc.scalar.activation(e[:], ga[:, 0, 0:Ck], EXP,
                                             scale=SCALE_HIGH)
                        hstate[f"ea{t}"] = e

                def high_softmax():
                    # SBUF-only chain -> Pool engine (keeps DVE free for exp)
                    ea = [hstate[f"ea{t}"] for t in range(T)]
                    s01 = wp.tile([128, Ck], F32)
                    nc.gpsimd.tensor_add(s01[:], ea[0][:], ea[1][:])
                    s23 = wp.tile([128, Ck], F32)
                    nc.gpsimd.tensor_add(s23[:], ea[2][:], ea[3][:])
                    ssum = wp.tile([128, Ck], F32)
                    nc.gpsimd.tensor_add(ssum[:], s01[:], s23[:])
                    rs = wp.tile([128, Ck], F32)
                    nc.vector.reciprocal(rs[:], ssum[:])
                    for t in range(T):
                        wt = wp.tile([128, Ck], BF16, name=f"wt{t}", tag=f"wt{t}")
                        nc.gpsimd.tensor_mul(wt[:], ea[t][:], rs[:])
                        hstate[f"wt{t}"] = wt

                def high_qvup():
                    qvup = qkps.tile([128, 2, 512], F32, name="qvup", tag="qk")
                    for c in range(NC_CHUNKS):
                        mm2b(qvup, qvT_t[:, c, :], uj_t[:, c, :, :],
                             start=(c == 0), stop=(c == NC_CHUNKS - 1))
                    qvup_bf = wp.tile([128, 2, MH], BF16)
                    nc.vector.tensor_copy(qvup_bf[:], qvup[:, :, 0:MH])
                    hstate["qvup_bf"] = qvup_bf

                def high_qo(t):
                    wt = hstate[f"wt{t}"]
                    qvup_bf = hstate["qvup_bf"]
                    qo = qkps.tile([128, 2, 512], F32, name=f"qo{t}", tag="qk")
                    mm2b(qo[0:Ck], wt[:, :], qvup_bf[:, :, :],
                         start=True, stop=True)
                    if QO_DMA_PSUM:
                        nc.sync.dma_start(
                            d_out.ap()[t * Ck:(t + 1) * Ck, :, :],
                            qo[0:Ck, :, 0:MH])
                    else:
                        qo_sb = wp.tile([Ck, 2, MH], BF16,
                                        name=f"qosb{t}", tag="qosb")
                        nc.vector.tensor_copy(qo_sb[:], qo[0:Ck, :, 0:MH])
                        nc.sync.dma_start(
                            d_out.ap()[t * Ck:(t + 1) * Ck, :, :], qo_sb[:])

                # placement is load-bearing: qvup's 10 back-to-back bf16
                # matmuls are the >=3.4us gap-free PE run that flips the HAM
                # clock gate to 8/8 at ~32-36us -- late enough that the chip
                # power arbiter grants the rest of the run at full clock.
                # Earlier (<~30us) flips get punished with chip-wide 4/8
                # clamps; later placement leaves most of the kernel at 1.2GHz.
                # qvup placement is load-bearing: its 10 back-to-back bf16
                # matmuls are a >=3.4us gap-free PE run -- the RE-FLIP
                # insurance for cores whose early warmup flip drops on an
                # unlucky HAM window phase. Do NOT move it into the warmup
                # seam (a DMA stall there leaves no recovery trigger).
                HIGH_AT = {
                    # all 4 ga blocks back-to-back right after the warmup
                    # seam: 20 dependency-free matmuls = 5.2us dense at 4/8,
                    # enough to re-flip the clock gate by itself if the
                    # warmup flip dropped on a bad window phase.
                    4: lambda: high_ga((0, 1, 2, 3)),
                    6: high_qvup,
                    7: high_softmax,
                    8: lambda: high_qo(0),
                    10: lambda: high_qo(1),
                    12: lambda: high_qo(2),
                    14: lambda: high_qo(3),
                }

                # ---------- PE warm-up: ramp DVFS while DMA streams ---------
                if WARMUP_MM > 0:
                    wub = wp.tile([128, 512], BF16)
                    nc.gpsimd.memset(wub[:], 1.0)
                    wu = qkps.tile([128, 2, 512], F32, name="warm", tag="qk")
                    for _ in range(WARMUP_MM):
                        nc.tensor.matmul(wu[:, 0, :], wub[:, 0:128],
                                         wub[:, :], start=True, stop=True)

                # ---------- software-pipelined low loop over n-tile pairs ---
                # denominator: half 0 on PE (DoubleRow ones matmul into PSUM),
                # half 1 on Pool (fp32 ping-pong accumulation of SBUF e8).
                dacc = [wp.tile([128, 2, MH], F32, name=f"dacc{x}") for x in (0, 1)]

                def emit_dn(e8, qq, sp):
                    nc.tensor.matmul(dn[:, 0, 0:MH], ones8[:], e8[:, :, 0, :],
                                     perf_mode=DR, start=(qq == 0), stop=sp)
                    if qq < DN_DVE_PAIRS:
                        # DVE fp32 ping-pong accumulation of the fp8 e tiles
                        # (PE relief while the clock gate still holds 4/8)
                        s_, dst = dacc[qq % 2], dacc[(qq + 1) % 2]
                        if qq == 0:
                            nc.vector.tensor_copy(dst[:], e8[:, :, 1, :])
                        else:
                            nc.vector.tensor_add(dst[:], s_[:], e8[:, :, 1, :])
                    else:
                        nc.tensor.matmul(
                            dn[:, 1, 0:MH], ones8[:], e8[:, :, 1, :],
                            perf_mode=DR, start=(qq == DN_DVE_PAIRS), stop=sp)

                def emit_avdn(e8, qq, dn_first=False):
                    st, sp = (qq == 0), (qq == NPAIR - 1)
                    mvk = mvT_t[:, 2 * qq:2 * qq + 2, :]   # (128, 2, 128)
                    if dn_first:
                        emit_dn(e8, qq, sp)
                        mm2b_dr(av, mvk, e8[:, :, :, :], start=st, stop=sp)
                    else:
                        mm2b_dr(av, mvk, e8[:, :, :, :], start=st, stop=sp)
                        emit_dn(e8, qq, sp)

                qkA, qkB = emit_qk(0), emit_qk(1)
                pend = []  # [(e8, qq), ...] awaiting av/dn (depth-2 lag)
                for qq in range(NPAIR):
                    if qq == 28 and WARM_MID > 0:
                        # second re-flip block: drops observed at ~41us (a
                        # thermal duty-cycle pattern as the chip heats) land
                        # after the qq20 block; this bounds their cost.
                        wu3 = qkps.tile([128, 2, 512], F32, name="warm3",
                                        tag="qk")
                        for _ in range(8):
                            nc.tensor.matmul(wu3[:, 0, :], wub[:, 0:128],
                                             wub[:, :], start=True, stop=True)
                    if qq == 20 and WARM_MID > 0:
                        # late re-flip insurance: by now every dense block
                        # (warmup, ga, qvup) has passed; a core whose clock
                        # gate dropped after them would otherwise finish the
                        # rest of the loop at 1.2 GHz (+20us). 8 back-to-back
                        # MMs re-flip it; cost on healthy cores ~1.7us.
                        wu2 = qkps.tile([128, 2, 512], F32, name="warm2",
                                        tag="qk")
                        for _ in range(8):
                            nc.tensor.matmul(wu2[:, 0, :], wub[:, 0:128],
                                             wub[:, :], start=True, stop=True)
                    if qq == WARM_AT and WARM_MID > 0:
                        # dense dummy-MM run flips the HAM gate to 8/8; the
                        # av/dn backlog held back below is then flushed
                        # back-to-back so the PE never gaps while the qk/exp
                        # pipeline respools (a gap here would drop the gate).
                        wub = wp.tile([128, 512], BF16)
                        nc.gpsimd.memset(wub[:], 1.0)
                        wu = qkps.tile([128, 2, 512], F32, name="warm", tag="qk")
                        for _ in range(WARM_MID):
                            nc.tensor.matmul(wu[:, 0, :], wub[:, 0:128],
                                             wub[:, :], start=True, stop=True)
                        while len(pend) > 2:
                            emit_avdn(*pend.pop(0))
                    if qq in HIGH_AT:
                        HIGH_AT[qq]()
                    e8 = epool.tile([128, 2, 2, MH], F8, name=f"e{qq}", tag="e")
                    exp_write(e8, 0, qkA, 2 * qq)
                    exp_write(e8, 1, qkB, 2 * qq + 1)
                    if qq + 1 < NPAIR:
                        qkA, qkB = emit_qk(2 * qq + 2), emit_qk(2 * qq + 3)
                    hold = WARM_MID > 0 and WARM_AT - 3 <= qq < WARM_AT
                    if len(pend) >= 2 and not hold:
                        emit_avdn(*pend.pop(0))
                    pend.append((e8, qq))
                for p in pend:
                    emit_avdn(*p, dn_first=True)

                # ---------- normalize + store memory rows -------------------
                mem_sb = wp.tile([128, 2, MH], BF16)
                r0 = T * Ck
                dn1 = wp.tile([128, MH], F32)
                if DN_DVE_PAIRS > 0:
                    # fold the DVE-accumulated prefix (both tile planes) into
                    # the PSUM-accumulated suffix of half 1
                    last = dacc[min(DN_DVE_PAIRS, NPAIR) % 2]
                    dn1p = wp.tile([128, MH], F32)
                    nc.vector.tensor_add(dn1p[:], last[:, 0, :], last[:, 1, :])
                    if DN_DVE_PAIRS < NPAIR:
                        nc.vector.tensor_add(dn1[:], dn1p[:], dn[:, 1, 0:MH])
                    else:
                        dn1 = dn1p
                    dn_half = [dn_h0, dn1[:]]
                else:
                    dn_half = [dn_h0, dn[:, 1, 0:MH]]
                if DIV_TAIL:
                    DIVOP = mybir.AluOpType.divide
                    for h in range(2):
                        nc.vector.tensor_tensor(
                            mem_sb[:, h, :], av[:, h, 0:MH], dn_half[h], DIVOP)
                        nc.sync.dma_start(
                            d_out.ap()[r0:r0 + Cv, h, :], mem_sb[:, h, :])
                elif DN_DVE_PAIRS == 0:
                    # single-shot: one reciprocal + one mul over both halves;
                    # bf16 output is small enough for one DMA per half
                    rcp_sb = wp.tile([128, 2, MH], F32)
                    rcp_scr = wp.tile([128, 2, MH], F32)
                    nc.vector.reciprocal_approx_accurate(
                        rcp_sb[:], dn[:, :, 0:MH], rcp_scr[:])
                    nc.vector.tensor_mul(
                        mem_sb[:], av[:, :, 0:MH], rcp_sb[:])
                    for h in range(2):
                        nc.sync.dma_start(
                            d_out.ap()[r0:r0 + Cv, h, :], mem_sb[:, h, :])
                else:
                    rcp_sb = wp.tile([128, 2, MH], F32)
                    rcp_scr = wp.tile([128, 2, MH], F32)
                    for h in range(2):
                        nc.vector.reciprocal_approx_accurate(
                            rcp_sb[:, h, :], dn_half[h], rcp_scr[:, h, :])
                        nc.vector.tensor_mul(
                            mem_sb[:, h, :], av[:, h, 0:MH], rcp_sb[:, h, :])
                        nc.sync.dma_start(
                            d_out.ap()[r0:r0 + Cv // 2, h, :],
                            mem_sb[0:Cv // 2, h, :])
                        nc.sync.dma_start(
                            d_out.ap()[r0 + Cv // 2:r0 + Cv, h, :],
                            mem_sb[Cv // 2:Cv, h, :])

    nc.compile()
    return nc


def _get_program():
    global _PROGRAM
    if _PROGRAM is None:
        _PROGRAM = _build_program()
    return _PROGRAM


def _prep_core_inputs(memory_keys, memory_values, query_value,
                      memory_keys_low, memory_values_low, query_key_low,
                      Ufull, b, j):
    bf = ml_dtypes.bfloat16
    f8 = ml_dtypes.float8_e4m3

    # ---- low branch (fp8)
    mk_cn = memory_keys_low[b].transpose(1, 0, 2, 3).reshape(Ck, NLOW)
    mk4 = np.ascontiguousarray(mk_cn.reshape(Ck, NT, 128)).astype(f8)

    mv_cn = memory_values_low[b].transpose(1, 0, 2, 3).reshape(Cv, NLOW)
    mvT = np.ascontiguousarray(
        mv_cn.reshape(Cv, NT, 128).transpose(2, 1, 0)
    ).astype(f8)  # (p, k, cv)

    qkl = query_key_low[b].reshape(Ck, MTOT)[:, j * MBLK:(j + 1) * MBLK]
    qkl2 = np.ascontiguousarray(qkl).astype(f8).reshape(Ck, 2, MH)

    # ---- high branch (bf16, zero-padded hw -> 640 = 5*128 chunks)
    gk = memory_keys[b].reshape(T, Ck, HW)
    gkp = np.zeros((T, Ck, HWP), np.float32)
    gkp[:, :, :HW] = gk
    gkT = np.ascontiguousarray(
        gkp.reshape(T, Ck, NC_CHUNKS, 128).transpose(3, 0, 2, 1)
    ).astype(bf)  # (p, t, c, k)

    gv = memory_values[b].reshape(T, Cv, HW)
    gvp = np.zeros((T, Cv, HWP), np.float32)
    gvp[:, :, :HW] = gv
    gvT = np.ascontiguousarray(
        gvp.reshape(T, Cv, NC_CHUNKS, 128).transpose(3, 0, 2, 1)
    ).astype(bf)  # (p, t, c, v)

    qv = query_value[b].reshape(Cv, HW)
    qvp = np.zeros((Cv, HWP), np.float32)
    qvp[:, :HW] = qv
    qvT = np.ascontiguousarray(
        qvp.reshape(Cv, NC_CHUNKS, 128).transpose(2, 1, 0)
    ).astype(bf)  # (p, c, v)

    ujf = np.zeros((HWP, MBLK), np.float32)
    ujf[:HW, :] = Ufull[:, j * MBLK:(j + 1) * MBLK]
    uj = np.ascontiguousarray(
        ujf.reshape(NC_CHUNKS, 128, MBLK).transpose(1, 0, 2)
    ).astype(bf).reshape(128, NC_CHUNKS, 2, MH)  # (p, c, h, 288)

    return {
        "qkl2": qkl2, "mk": mk4, "mvT": mvT,
        "gkT": gkT, "gvT": gvT, "qvT": qvT, "uj": uj,
    }


def kernel(memory_keys, memory_values, query_value,
           memory_keys_low, memory_values_low, query_key_low):
    memory_keys = np.asarray(memory_keys, dtype=np.float32)
    memory_values = np.asarray(memory_values, dtype=np.float32)
    query_value = np.asarray(query_value, dtype=np.float32)
    memory_keys_low = np.asarray(memory_keys_low, dtype=np.float32)
    memory_values_low = np.asarray(memory_values_low, dtype=np.float32)
    query_key_low = np.asarray(query_key_low, dtype=np.float32)

    Ufull = _build_upsample_full()
    nc = _get_program()

    in_maps = []
    for core in range(8):
        b, j = core // 4, core % 4
        in_maps.append(_prep_core_inputs(
            memory_keys, memory_values, query_value,
            memory_keys_low, memory_values_low, query_key_low, Ufull, b, j))

    trace = os.environ.get("KERNEL_TRACE", "0") == "1"
    kwargs = {}
    if trace and os.environ.get("KERNEL_TRACE_DIR"):
        os.makedirs(os.environ["KERNEL_TRACE_DIR"], exist_ok=True)
        kwargs["tmpdir"] = os.environ["KERNEL_TRACE_DIR"]
    res = bass_utils.run_bass_kernel_spmd(
        nc, in_maps, core_ids=list(range(8)), trace=trace, **kwargs
    )
    LAST_PERF.clear()
    LAST_PERF.update(
        exec_time_ns=res.exec_time_ns,
        mean_exec_time_ns=getattr(res, "mean_exec_time_ns", None),
        max_exec_time_core_id=getattr(res, "max_exec_time_core_id", None),
        per_core_scope_times=getattr(res, "per_core_scope_times", None),
        trace=getattr(res, "instructions_and_trace", None),
    )

    out = np.empty((B, T * Ck + Cv, Hl, Wl), np.float32)
    for core in range(8):
        b, j = core // 4, core % 4
        blk = np.asarray(res.results[core]["out"], dtype=np.float32)
        blk = blk.reshape(T * Ck + Cv, MBLK)
        out[b, :, 12 * j:12 * (j + 1), :] = blk.reshape(T * Ck + Cv, 12, Wl)
    return out
